# revision 1
# baseline (speedup 1.0000x reference)
"""GATv2Conv multi-head kernel for 8 trn2 NeuronCores (Bass/Tile).

Strategy (edge/src sharding, host-expanded dst features):
  - The softmax-surrogate ratio h'/rows_sum cancels the entire s0/X0/W0 path
    except through the global min-max scale sigma = 1/(mx-mn).
  - Host expands X[dst].T per core (static index prep). Device computes
    per-edge X1' rows with fp16 matmuls (lhsT = X[dst].T tile, rhs = W1.T),
    leaky via ACT Prelu(alpha=0.2), per-edge s1 via DVE mult+reduce.
  - sweep1: per-edge s1 + global min/max of (s0[src]+s1[dst]) -> AllReduce.
  - sweep2: attn = exp(sigma*s1), folded into segment-sum matmuls
    (lhsT = scattered attn [128 edges, 32 (h,m)], rhs = leaky'd features),
    rows_sum via a ones-column matmul, division on device.
  - Output returned in a partition-major raw layout; host reindexes.
"""
import sys
if '/opt/trn_rl_repo' not in sys.path:
    sys.path.insert(0, '/opt/trn_rl_repo')

import numpy as np

# ---- problem constants (hardcoded per contest rules) ----
N = 100000
E = 1600000
IN = 128
D = 32
H = 4
DEG = 16
ALPHA = 0.2
N_CORES = 8

ET = 1568                 # edge tiles per core (128 edges each), padded
EDGES_LOC = ET * 128      # 200704 edge slots per core
NODES_LOC = ET * 8        # 12544 node slots per core
BLOCKS = ET // 4          # 392 blocks of 512 edges
STS = BLOCKS // 8         # 49 supertiles of 4096 edges

_PROG_CACHE = {}


def _build_program():
    import os
    _STOP = int(os.environ.get("K_STOP_AFTER", "99"))
    import concourse.bass as bass
    import concourse.tile as tile
    from concourse import bacc, mybir

    F32 = mybir.dt.float32
    F16 = mybir.dt.float16
    AF = mybir.ActivationFunctionType
    ALU = mybir.AluOpType

    nc = bacc.Bacc("TRN2", target_bir_lowering=False, debug=False,
                   enable_asserts=False, num_devices=N_CORES)

    # ---------------- I/O ----------------
    xdt = nc.dram_tensor("xdt", [128, EDGES_LOC], F16, kind="ExternalInput").ap()
    xst = nc.dram_tensor("xst", [128, NODES_LOC], F16, kind="ExternalInput").ap()
    w0t = nc.dram_tensor("w0t", [128, 128], F16, kind="ExternalInput").ap()
    w1t = nc.dram_tensor("w1t", [128, 128], F16, kind="ExternalInput").ap()
    a_mat = nc.dram_tensor("a_mat", [128, 4], F16, kind="ExternalInput").ap()
    a_rep = nc.dram_tensor("a_rep", [128, 128], F16, kind="ExternalInput").ap()
    mask8 = nc.dram_tensor("mask8", [128, 8], F16, kind="ExternalInput").ap()
    hp_raw = nc.dram_tensor("hp_raw", [128, BLOCKS * 128], F32,
                            kind="ExternalOutput").ap()

    # ---------------- internal DRAM ----------------
    s0x = nc.dram_tensor("s0x", [EDGES_LOC, 4], F32)
    msg_d = nc.dram_tensor("msg_d", [128, EDGES_LOC], F16)
    s1d = nc.dram_tensor("s1d", [128, STS * 128], F32)
    mmd = nc.dram_tensor("mmd", [128, 8], F32)
    mm_loc = nc.dram_tensor("mm_loc", [1, 8], F32)
    mm_glob = nc.dram_tensor("mm_glob", [1, 8], F32, addr_space="Shared")
    sig_d = nc.dram_tensor("sig_d", [1, 4], F32)

    with tile.TileContext(nc) as tc:
        with tc.tile_pool(name="const", bufs=1) as constp:
            w1t_t = constp.tile([128, 128], F16)
            nc.sync.dma_start(w1t_t[:], w1t[:])
            w0t_t = constp.tile([128, 128], F16)
            nc.sync.dma_start(w0t_t[:], w0t[:])
            a_mat_t = constp.tile([128, 4], F16)
            nc.sync.dma_start(a_mat_t[:], a_mat[:])
            a_rep_t = constp.tile([128, 128], F16)
            nc.sync.dma_start(a_rep_t[:], a_rep[:])
            mask8_t = constp.tile([128, 8], F16)
            nc.sync.dma_start(mask8_t[:], mask8[:])
            ones_t = constp.tile([128, 1], F16)
            nc.vector.memset(ones_t[:], 1.0)

            # ---------------- phase 0: s0 per own node (node-major) ----------------
            with tc.tile_pool(name="p0", bufs=3) as p0, \
                 tc.tile_pool(name="p0ps", bufs=2, space="PSUM") as p0ps:
                for i in range(NODES_LOC // 128):
                    xs_t = p0.tile([128, 128], F16, tag="xs")
                    nc.sync.dma_start(xs_t[:], xst[:, i*128:(i+1)*128])
                    ps0 = p0ps.tile([128, 128], F32, tag="ps0")
                    nc.tensor.matmul(out=ps0[:], lhsT=xs_t[:], rhs=w0t_t[:],
                                     start=True, stop=True)
                    lk = p0.tile([128, 128], F16, tag="lk")
                    nc.scalar.activation(lk[:], ps0[:], AF.Prelu, alpha=ALPHA)
                    tmp0 = p0.tile([128, 128], F16, tag="tmp0")
                    nc.vector.tensor_tensor(out=tmp0[:], in0=lk[:],
                                            in1=a_rep_t[:], op=ALU.mult)
                    s0blk = p0.tile([128, 4], F32, tag="s0blk")
                    nc.vector.tensor_reduce(
                        out=s0blk[:],
                        in_=tmp0[:].rearrange("p (h d) -> p h d", d=32),
                        axis=mybir.AxisListType.X, op=ALU.add)
                    # expand x16 in SBUF, then contiguous store (256B runs)
                    s0rep = p0.tile([128, 64], F32, tag="s0rep")
                    nc.vector.tensor_copy(
                        s0rep[:],
                        s0blk[:].unsqueeze(1).to_broadcast([128, 16, 4]))
                    nc.sync.dma_start(
                        bass.AP(s0x, i * 128 * 64, [[64, 128], [1, 64]]),
                        s0rep[:])

            # ---------------- sweep 1 ----------------
            if _STOP >= 1:
                with tc.tile_pool(name="s1xq", bufs=2) as s1xq, \
                     tc.tile_pool(name="s1w", bufs=3) as s1w, \
                     tc.tile_pool(name="s1ps", bufs=4, space="PSUM") as s1ps, \
                     tc.tile_pool(name="s1mm", bufs=1) as s1mmp:
                    rmx = s1mmp.tile([128, 8], F32)
                    nc.vector.memset(rmx[:, 0:4], -1e30)
                    nc.vector.memset(rmx[:, 4:8], -1e30)  # holds -min
                    for st in range(STS):
                        e0 = st * 4096
                        xq = s1xq.tile([128, 4096], F16, tag="xq")
                        nc.sync.dma_start(xq[:], xdt[:, e0:e0 + 4096])
                        s0e = s1w.tile([128, 128], F32, tag="s0e")
                        nc.sync.dma_start(
                            s0e[:],
                            bass.AP(s0x, e0 * 4, [[4, 128], [512, 32], [1, 4]]))
                        msgst = s1w.tile([128, 4096], F16, tag="msgst")
                        s1st = s1w.tile([128, 128], F32, tag="s1st")
                        attst = s1w.tile([128, 128], F32, tag="attst")
                        for b in range(8):
                            ps = s1ps.tile([128, 512], F32, tag="mm1")
                            for j in range(4):
                                nc.tensor.matmul(
                                    out=ps[:, j*128:(j+1)*128],
                                    lhsT=xq[:, (b*4+j)*128:(b*4+j+1)*128],
                                    rhs=w1t_t[:], start=True, stop=True)
                            nc.scalar.activation(msgst[:, b*512:(b+1)*512], ps[:],
                                                 AF.Prelu, alpha=ALPHA)
                            tmp = s1w.tile([128, 512], F16, tag="tmp")
                            nc.vector.tensor_tensor(
                                out=tmp[:],
                                in0=msgst[:, b*512:(b+1)*512].rearrange(
                                    "p (t c) -> p t c", c=128),
                                in1=a_rep_t[:].unsqueeze(1).to_broadcast([128, 4, 128]),
                                op=ALU.mult)
                            nc.vector.tensor_reduce(
                                out=s1st[:, b*16:(b+1)*16],
                                in_=tmp[:].rearrange("p (jh d) -> p jh d", d=32),
                                axis=mybir.AxisListType.X, op=ALU.add)
                            nc.vector.tensor_add(attst[:, b*16:(b+1)*16],
                                                 s1st[:, b*16:(b+1)*16],
                                                 s0e[:, b*16:(b+1)*16])
                        # minmax accumulate over supertile
                        red = s1w.tile([128, 8], F32, tag="red")
                        att3 = attst[:].rearrange("p (bj h) -> p h bj", h=4)
                        nc.vector.tensor_reduce(out=red[:, 0:4], in_=att3,
                                                axis=mybir.AxisListType.X,
                                                op=ALU.max)
                        nc.vector.tensor_reduce(out=red[:, 4:8], in_=att3,
                                                axis=mybir.AxisListType.X,
                                                op=ALU.min)
                        nc.vector.tensor_scalar(out=red[:, 4:8], in0=red[:, 4:8],
                                                scalar1=-1.0, scalar2=None,
                                                op0=ALU.mult)
                        nc.vector.tensor_max(rmx[:], rmx[:], red[:])
                        nc.sync.dma_start(msg_d[:, e0:e0 + 4096], msgst[:])
                        nc.sync.dma_start(s1d[:, st*128:(st+1)*128], s1st[:])
                    nc.sync.dma_start(mmd[:], rmx[:])

            # ---------------- minmax allreduce + sigma ----------------
            if _STOP >= 2:
                with tc.tile_pool(name="mmp", bufs=1) as mmp:
                    redin = mmp.tile([1, 1024], F32)
                    nc.sync.dma_start(redin[:],
                                      bass.AP(mmd, 0, [[0, 1], [1, 1024]]))
                    red8 = mmp.tile([1, 8], F32)
                    nc.vector.tensor_reduce(
                        out=red8[:],
                        in_=redin[:].rearrange("q (p v) -> q v p", v=8),
                        axis=mybir.AxisListType.X, op=ALU.max)
                    nc.sync.dma_start(mm_loc[:], red8[:])
                    nc.gpsimd.collective_compute(
                        "AllReduce", ALU.max,
                        replica_groups=[list(range(N_CORES))],
                        ins=[mm_loc[:]], outs=[mm_glob[:]])
                    garr = mmp.tile([1, 8], F32)
                    nc.sync.dma_start(garr[:], mm_glob[:])
                    rng_t = mmp.tile([1, 4], F32)
                    nc.vector.tensor_add(rng_t[:], garr[:, 0:4], garr[:, 4:8])
                    sig_t = mmp.tile([1, 4], F32)
                    nc.vector.reciprocal(sig_t[:], rng_t[:])
                    nc.sync.dma_start(sig_d[:], sig_t[:])

            # ---------------- sweep 2 ----------------
            if _STOP >= 3:
                with tc.tile_pool(name="s2w", bufs=2) as s2w, \
                     tc.tile_pool(name="s2ps", bufs=4, space="PSUM") as s2ps:
                    sigb = constp.tile([128, 4], F32)
                    nc.sync.dma_start(sigb[:], bass.AP(sig_d, 0, [[0, 128], [1, 4]]))
                    for st in range(STS):
                        e0 = st * 4096
                        msgq = s2w.tile([128, 4096], F16, tag="msgq")
                        nc.sync.dma_start(msgq[:], msg_d[:, e0:e0 + 4096])
                        s1t = s2w.tile([128, 128], F32, tag="s1t")
                        nc.sync.dma_start(s1t[:], s1d[:, st*128:(st+1)*128])
                        atw = s2w.tile([128, 128], F32, tag="atw")
                        nc.vector.tensor_tensor(
                            out=atw[:],
                            in0=s1t[:].rearrange("p (bj h) -> p bj h", h=4),
                            in1=sigb[:].unsqueeze(1).to_broadcast([128, 32, 4]),
                            op=ALU.mult)
                        attn16 = s2w.tile([128, 128], F16, tag="attn16")
                        nc.scalar.activation(attn16[:], atw[:], AF.Exp)
                        sa = s2w.tile([128, 1024], F16, tag="sa")
                        nc.vector.tensor_tensor(
                            out=sa[:],
                            in0=attn16[:].unsqueeze(2).to_broadcast([128, 128, 8]),
                            in1=mask8_t[:].unsqueeze(1).to_broadcast([128, 128, 8]),
                            op=ALU.mult)
                        stg = s2w.tile([128, 8 * 129], F32, tag="stg")
                        for b in range(8):
                            psS = s2ps.tile([128, 132], F32, tag="seg")
                            for j in range(4):
                                lhs = sa[:, (b*4+j)*32:(b*4+j+1)*32]
                                nc.tensor.matmul(
                                    out=psS[32*j:32*j+32, 0:128], lhsT=lhs,
                                    rhs=msgq[:, (b*4+j)*128:(b*4+j+1)*128],
                                    start=True, stop=True,
                                    tile_position=(0, 32*j))
                                nc.tensor.matmul(
                                    out=psS[32*j:32*j+32, 128:129], lhsT=lhs,
                                    rhs=ones_t[:], start=True, stop=True,
                                    tile_position=(0, 32*j))
                            nc.vector.tensor_copy(stg[:, b*129:(b+1)*129],
                                                  psS[:, 0:129])
                        rcp = s2w.tile([128, 8], F32, tag="rcp")
                        nc.vector.reciprocal(
                            rcp[:], stg[:].rearrange("p (b c) -> p b c", c=129)[:, :, 128:129])
                        divq = s2w.tile([128, 1024], F32, tag="divq")
                        nc.vector.tensor_tensor(
                            out=divq[:],
                            in0=stg[:].rearrange("p (b c) -> p b c", c=129)[:, :, 0:128],
                            in1=rcp[:].unsqueeze(2).to_broadcast([128, 8, 128]),
                            op=ALU.mult)
                        nc.sync.dma_start(hp_raw[:, st*1024:(st+1)*1024], divq[:])

    nc.compile()
    return nc


def _preprocess(X, W0, W1, a0, edge_src, column_index):
    """Host-side index-only data prep. Returns per-core input maps."""
    Xf16t = np.ascontiguousarray(X.T.astype(np.float16))      # [128, N]
    w0t = np.ascontiguousarray(W0.T.astype(np.float16))
    w1t = np.ascontiguousarray(W1.T.astype(np.float16))
    a_vec = a0.reshape(H * D).astype(np.float16)
    a_mat = np.zeros((128, 4), np.float16)
    for h in range(H):
        a_mat[h*D:(h+1)*D, h] = a_vec[h*D:(h+1)*D]
    a_rep = np.ascontiguousarray(np.tile(a_vec[None, :], (128, 1)))
    mask8 = np.zeros((128, 8), np.float16)
    for p in range(128):
        mask8[p, p // 16] = 1.0

    # per-core tile counts: 12500 global tiles, cores 0-3 get 1563, 4-7 get 1562
    t_real = [1563, 1563, 1563, 1563, 1562, 1562, 1562, 1562]
    ins = []
    meta = []
    e_base = 0
    for c in range(N_CORES):
        tr = t_real[c]
        n_edges = tr * 128
        dst = column_index[e_base:e_base + n_edges].astype(np.int64)
        pad_edges = EDGES_LOC - n_edges
        # pad with duplicates of the core's first tile(s)
        dst_pad = np.concatenate([dst, np.resize(dst[:128], pad_edges)])
        xdt = np.ascontiguousarray(Xf16t[:, dst_pad])          # [128, EDGES_LOC]

        nb = e_base // DEG
        r_nodes = tr * 8
        own = np.arange(nb, nb + r_nodes)
        pad_nodes = NODES_LOC - r_nodes
        own_pad = np.concatenate([own, nb + (np.arange(pad_nodes) % 8)])
        xst = np.ascontiguousarray(Xf16t[:, own_pad])          # [128, NODES_LOC]

        ins.append({"xdt": xdt, "xst": xst, "w0t": w0t, "w1t": w1t,
                    "a_mat": a_mat, "a_rep": a_rep, "mask8": mask8})
        meta.append((nb, r_nodes))
        e_base += n_edges
    return ins, meta


def _extract(results, meta):
    out = np.empty((N, H, D), np.float32)
    # hp_raw [128, BLOCKS*128]: partition 32j+8h+m, col blk*128 + 32h + d
    #   -> node_loc = blk*32 + j*8 + m, head h, feat d
    for c, res in enumerate(results):
        nb, r_nodes = meta[c]
        raw = res["hp_raw"].reshape(128, BLOCKS, 128)          # [p, blk, c]
        raw = raw.reshape(4, 4, 8, BLOCKS, 4, 32)              # [j, h, m, blk, h2, d]
        # select h2 == h; reorder to [blk, j, m, h, d]
        idx_h = np.arange(4)
        sel = raw[:, idx_h, :, :, idx_h, :]                    # [h, j, m, blk, d]
        sel = sel.transpose(3, 1, 2, 0, 4).reshape(NODES_LOC, H, D)
        out[nb:nb + r_nodes] = sel[:r_nodes]
    return out


def _reference_fallback(X, W0, W1, a0, edge_src, column_index):
    X0 = X @ W0.T
    X0 = np.where(X0 > 0, X0, ALPHA * X0).reshape(N, H, D).transpose(1, 0, 2)
    X1 = X @ W1.T
    X1 = np.where(X1 > 0, X1, ALPHA * X1).reshape(N, H, D).transpose(1, 0, 2)
    a = a0[:, 0, :]
    s0 = np.einsum('hnd,hd->hn', X0, a)
    s1 = np.einsum('hnd,hd->hn', X1, a)
    att = s0[:, edge_src] + s1[:, column_index]
    mx = att.max(axis=1, keepdims=True)
    mn = att.min(axis=1, keepdims=True)
    att = np.exp((att - mn) / (mx - mn))
    rows_sum = np.zeros((N, H), np.float32)
    np.add.at(rows_sum, edge_src, att.T)
    msg = att.T[:, :, None] * X1[:, column_index, :].transpose(1, 0, 2)
    hp = np.zeros((N, H, D), np.float32)
    np.add.at(hp, edge_src, msg)
    return (hp / rows_sum[:, :, None]).astype(np.float32)


def kernel(X, W0, W1, a0, edge_src, column_index):
    X = np.asarray(X, np.float32)
    W0 = np.asarray(W0, np.float32)
    W1 = np.asarray(W1, np.float32)
    a0 = np.asarray(a0, np.float32).reshape(H, 1, D)
    edge_src = np.asarray(edge_src, np.int32)
    column_index = np.asarray(column_index, np.int32)

    uniform = (X.shape == (N, IN) and column_index.shape == (E,)
               and np.array_equal(edge_src,
                                  np.repeat(np.arange(N, dtype=np.int32), DEG)))
    if not uniform:
        return _reference_fallback(X, W0, W1, a0, edge_src, column_index)

    from concourse.bass_utils import run_bass_kernel_spmd
    if "nc" not in _PROG_CACHE:
        _PROG_CACHE["nc"] = _build_program()
    nc = _PROG_CACHE["nc"]

    ins, meta = _preprocess(X, W0, W1, a0, edge_src, column_index)
    res = run_bass_kernel_spmd(nc, ins, core_ids=list(range(N_CORES)))
    return _extract(res.results, meta)

